# revision 25
# baseline (speedup 1.0000x reference)
"""TRN2 Bass kernel for nn_MaskingModule (topk entropy-KDE masking).

Self-contained: builds a Bass/Tile kernel, shards batch*seq across 8
NeuronCores (2 cores per batch sample, 256 rows each), runs via
run_bass_kernel_spmd, reassembles full outputs.

Returns (x_masked [4,128,1024] f32, mask [4,512] f32, ids_restore [4,512] i32).

Numerics are engineered to track the f32 reference bit-closely:
- nv = (v - vmin)/R via reciprocal + one Newton step (matches correctly
  rounded divide to <=1ulp)
- broadcast of nv to 128 partitions via exact 3-part bf16 split matmul
  (truncation split: 8+8+8 mantissa bits, products with one-hot exact,
  PSUM f32 reconstruction bit-exact)
- resid^2 via ACT Square with per-partition bins bias (exact)
- kern via ACT Exp (scale folded, ~3.4e-6 rms LUT err)
- 768-sums chunked 4x192 L->R f32 (DVE)
- entropy-stage reductions via exact f32 matmuls with mean-shift trick
"""
import os
import numpy as np

import concourse.bass as bass
import concourse.bacc as bacc
import concourse.mybir as mybir
import concourse.tile as tile
from concourse.bass import IndirectOffsetOnAxis
import concourse.bass_utils as _bu
from concourse.bass_utils import run_bass_kernel_spmd



F32 = mybir.dt.float32
I32 = mybir.dt.int32
BF16 = mybir.dt.bfloat16
AF = mybir.ActivationFunctionType
OP = mybir.AluOpType
AX = mybir.AxisListType

N, L, D, P = 4, 512, 1024, 768
LK = 128              # len_keep
RPC = 256             # rows per core
NB = 64               # bins

# --- constants replicating the reference's f32 arithmetic ---
BINS = np.arange(NB, dtype=np.float32) * (np.float32(1.0) / np.float32(63.0))
S_INV = np.float32(1.0) / np.float32(0.01)                      # fl(1/sigma)
C_EXP = float(np.float32(-0.5 * np.float64(S_INV) ** 2))        # exp scale
INV_P = float(np.float32(1.0) / np.float32(768.0))
C1 = np.float32(0.025)     # pdf mean-shift (64*C1 exact in f32)
C2 = np.float32(0.0586)    # prod mean-shift
C1_64 = float(np.float64(C1) * 64)
C2_64 = float(np.float64(C2) * 64)


def _make_sel6():
    # sel6[s2] [64,128] bf16: rows 6*s2+{0,1,2} (A parts) -> cols 0..63,
    # rows 6*s2+{3,4,5} (B parts) -> cols 64..127. One matmul contracts
    # hi+mid+lo for both rows of a tile.
    import ml_dtypes
    sels = np.zeros((8, 64, 128), np.float32)
    for s2 in range(8):
        sels[s2, 6 * s2:6 * s2 + 3, :64] = 1.0
        sels[s2, 6 * s2 + 3:6 * s2 + 6, 64:] = 1.0
    block = sels.transpose(1, 0, 2).reshape(64, 8 * 128)
    return np.tile(block, (2, 1)).astype(ml_dtypes.bfloat16)   # [128, 1024]


def _make_masks():
    # maskf [128, 2] f32: rows<64 -> col0, rows>=64 -> col1 (entropy-stage sums)
    mk = np.zeros((128, 2), np.float32)
    mk[:64, 0] = 1.0
    mk[64:, 1] = 1.0
    # maskT [2, 128] f32 for broadcasting [2,x]->[128,x]
    return mk, mk.T.copy()


_NC_CACHE = {}
LAST_RESULT = {}


def _build():
    if "nc" in _NC_CACHE:
        return _NC_CACHE["nc"]
    nc = bacc.Bacc("TRN2", target_bir_lowering=False, debug=False, num_devices=8)

    ipat = nc.dram_tensor("ipat", [128, 2 * P], F32, kind="ExternalInput")
    xrows = nc.dram_tensor("xrows", [128, 2 * D], F32, kind="ExternalInput")
    hbase = nc.dram_tensor("hbase", [1, 1], F32, kind="ExternalInput")  # h*256
    gimg = nc.dram_tensor("gimg", [128, 12288], F32, kind="ExternalInput")

    sel_d = nc.inline_tensor(_make_sel6(), name="sel_d")
    ebias_d = nc.inline_tensor(
        BINS[np.arange(128) % NB].reshape(128, 1).astype(np.float32), name="ebias_d")
    mk, mkT = _make_masks()
    mask_d = nc.inline_tensor(mk, name="mask_d")
    maskT_d = nc.inline_tensor(mkT, name="maskT_d")
    ones1_d = nc.inline_tensor(np.ones((1, 128), np.float32), name="ones1_d")

    xm_out = nc.dram_tensor("xm_out", [128, D], F32, kind="ExternalOutput")
    idr_out = nc.dram_tensor("idr_out", [1, RPC], I32, kind="ExternalOutput")
    mask_out = nc.dram_tensor("mask_out", [1, RPC], F32, kind="ExternalOutput")
    ent_out = nc.dram_tensor("ent_out", [2, 128], F32, kind="ExternalOutput")
    pkb_out = nc.dram_tensor("pkb_out", [128, 768], BF16, kind="ExternalOutput")
    sq0_out = nc.dram_tensor("sq0_out", [128, 1536], F32, kind="ExternalOutput")

    ag_in = nc.dram_tensor("ag_in", [1, RPC], F32, kind="Internal")
    ag_out = nc.dram_tensor("ag_out", [1, L], F32, kind="Internal")

    with tile.TileContext(nc) as tc:
        with (
            tc.tile_pool(name="cst", bufs=1) as cst,
            tc.tile_pool(name="work", bufs=1) as wk,
            tc.tile_pool(name="sq", bufs=2) as sqp,
            tc.tile_pool(name="kern", bufs=2) as kp,
            tc.tile_pool(name="p4", bufs=2) as pp,
            tc.tile_pool(name="ps", bufs=2, space="PSUM") as ps,
            tc.tile_pool(name="pse", bufs=1, space="PSUM") as pse,
        ):
            # ---- load inputs & constants ----
            t_ip = cst.tile([128, 2 * P], F32, name="t_ip")
            nc.sync.dma_start(t_ip[:], ipat.ap())
            t_sel = cst.tile([128, 1024], BF16, name="t_sel")
            nc.sync.dma_start(t_sel[:], sel_d.ap())
            t_eb = cst.tile([128, 1], F32, name="t_eb")
            nc.sync.dma_start(t_eb[:], ebias_d.ap())
            t_mask = cst.tile([128, 2], F32, name="t_mask")
            nc.sync.dma_start(t_mask[:], mask_d.ap())
            t_maskT = cst.tile([2, 128], F32, name="t_maskT")
            nc.sync.dma_start(t_maskT[:], maskT_d.ap())
            t_ones1 = cst.tile([1, 128], F32, name="t_ones1")
            nc.sync.dma_start(t_ones1[:], ones1_d.ap())
            t_x = cst.tile([128, 2 * D], F32, name="t_x")
            nc.sync.dma_start(t_x[:], xrows.ap())
            t_hb = cst.tile([1, 1], F32, name="t_hb")
            nc.sync.dma_start(t_hb[:], hbase.ap())

            # ---- global min/max: every core reduces ALL of img_pat ----
            # (f32 min/max is exact in any order; ~25us DMA-pipelined beats
            # the ~55us AllReduce round trip)
            t_pm = wk.tile([128, 16], F32, name="t_pm")
            for k in range(8):
                t_gik = wk.tile([128, 1536], F32, name=f"t_gi{k}", tag="gik", bufs=3)
                nc.sync.dma_start(t_gik[:], gimg.ap()[:, 1536 * k:1536 * (k + 1)])
                nc.vector.tensor_reduce(t_pm[:, k:k + 1], t_gik[:], axis=AX.X, op=OP.max)
                nc.vector.tensor_reduce(t_pm[:, 8 + k:9 + k], t_gik[:], axis=AX.X, op=OP.min)
            t_mm2 = wk.tile([128, 2], F32, name="t_mm2")
            nc.vector.tensor_reduce(t_mm2[:, 0:1], t_pm[:, 0:8], axis=AX.X, op=OP.max)
            t_rmin = wk.tile([128, 1], F32, name="t_rmin")
            nc.vector.tensor_reduce(t_rmin[:], t_pm[:, 8:16], axis=AX.X, op=OP.min)
            nc.vector.tensor_scalar(t_mm2[:, 1:2], t_rmin[:], -1.0, None, OP.mult)
            import concourse.bass_isa as bass_isa
            t_mmr = wk.tile([128, 2], F32, name="t_mmr")
            nc.gpsimd.partition_all_reduce(
                t_mmr[:], t_mm2[:], channels=128, reduce_op=bass_isa.ReduceOp.max)

            # derive [negmin, R, recipR] on partition 0, broadcast to 128
            t_pack = wk.tile([1, 3], F32, name="t_pack")
            nc.vector.tensor_copy(t_pack[:, 0:1], t_mmr[0:1, 1:2])      # negmin
            nc.vector.tensor_tensor(t_pack[:, 1:2], t_mmr[0:1, 0:1], t_mmr[0:1, 1:2], OP.add)  # R
            nc.vector.reciprocal(t_pack[:, 2:3], t_pack[:, 1:2])        # r
            t_cb = wk.tile([128, 3], F32, name="t_cb")
            nc.gpsimd.partition_broadcast(t_cb[:], t_pack[:], channels=128)

            # ---- Newton nv (negated): nvneg = (rem*r) - q0 ----
            t_d = wk.tile([128, 2 * P], F32, name="t_d")
            nc.vector.tensor_scalar(t_d[:], t_ip[:], t_cb[:, 0:1], None, OP.add)
            t_q0 = wk.tile([128, 2 * P], F32, name="t_q0")
            nc.vector.tensor_scalar(t_q0[:], t_d[:], t_cb[:, 2:3], None, OP.mult)
            t_rem = wk.tile([128, 2 * P], F32, name="t_rem")
            nc.vector.scalar_tensor_tensor(
                t_rem[:], t_q0[:], t_cb[:, 1:2], t_d[:], op0=OP.mult, op1=OP.subtract)
            t_nvn = wk.tile([128, 2 * P], F32, name="t_nvn")
            nc.vector.scalar_tensor_tensor(
                t_nvn[:], t_rem[:], t_cb[:, 2:3], t_q0[:], op0=OP.mult, op1=OP.subtract)

            # ---- exact 3-part truncation split to bf16 ----
            t_hi32 = wk.tile([128, 2 * P], F32, name="t_hi32")
            nc.vector.tensor_scalar(
                t_hi32[:].bitcast(mybir.dt.uint32), t_nvn[:].bitcast(mybir.dt.uint32),
                0xFFFF0000, None, OP.bitwise_and)
            t_r1 = wk.tile([128, 2 * P], F32, name="t_r1")
            nc.vector.tensor_tensor(t_r1[:], t_nvn[:], t_hi32[:], OP.subtract)
            t_mid32 = wk.tile([128, 2 * P], F32, name="t_mid32")
            nc.vector.tensor_scalar(
                t_mid32[:].bitcast(mybir.dt.uint32), t_r1[:].bitcast(mybir.dt.uint32),
                0xFFFF0000, None, OP.bitwise_and)
            t_hi = wk.tile([128, 2 * P], BF16, name="t_hi")
            t_mid = wk.tile([128, 2 * P], BF16, name="t_mid")
            t_lo = wk.tile([128, 2 * P], BF16, name="t_lo")
            nc.vector.tensor_copy(t_hi[:], t_hi32[:])
            nc.vector.tensor_copy(t_mid[:], t_mid32[:])
            nc.vector.tensor_tensor(t_lo[:], t_r1[:], t_mid32[:], OP.subtract)

            # ---- interleave parts by partition: window (parity, blk) holds
            # 8 tiles; tile slot s2 occupies partitions 6*s2..6*s2+5 =
            # (hiA,midA,loA,hiB,midB,loB); one K=64 matmul sums all parts.
            t_pkb = []
            for blk in range(8):
                tb = cst.tile([128, 768], BF16, name=f"t_pkb{blk}", tag=f"pkb{blk}")
                # partitions 48-63/112-127 are unused by the one-hot lhsT but
                # must be finite: 0*NaN = NaN would poison the PSUM.
                nc.gpsimd.memset(tb[:], 0.0)
                t_pkb.append(tb)
            parts3 = (t_hi, t_mid, t_lo)
            for ci in range(2):
                for j in range(4):
                    blk = ci * 4 + j
                    for parity in range(2):
                        for kpart in range(3):
                            sp = 16 * (parity + 2 * j)
                            b0 = 64 * parity + kpart
                            nc.sync.dma_start(
                                t_pkb[blk][b0:b0 + 46:3, 0:768],
                                parts3[kpart][sp:sp + 16, ci * P:ci * P + 768])

            # ---- main loop: 32 groups x 2 tiles x 2 rows ----
            nc.sync.dma_start(pkb_out.ap(), t_pkb[0][:])
            t_stats = cst.tile([128, 128], F32, name="t_stats")
            for g in range(64):
                # tiles 2g (rows 4g,4g+1) and 2g+1 (rows 4g+2,4g+3)
                pnv = ps.tile([128, 1536], F32, tag="pnv", name=f"pnv{g}")
                for half in range(2):           # A: psum cols 0:768, B: 768:1536
                    t = 2 * g + half
                    ci = t // 64
                    qq = (t % 64) // 8
                    s2 = t % 8
                    parity = qq % 2
                    blk = ci * 4 + qq // 2
                    lhs_ap = t_sel[64 * parity:64 * parity + 64,
                                   s2 * 128:(s2 + 1) * 128]
                    rb = t_pkb[blk]
                    base = half * 768
                    # regions must not cross PSUM bank boundaries (512 f32)
                    regions = ((0, 512), (512, 768)) if half == 0 else ((0, 256), (256, 768))
                    for (c0, c1) in regions:
                        nc.tensor.matmul(
                            pnv[:, base + c0:base + c1], lhs_ap,
                            rb[64 * parity:64 * parity + 64, c0:c1],
                            start=True, stop=True,
                        )
                t_sq = sqp.tile([128, 1536], F32, tag="sq", name=f"sq{g}")
                if g % 4 == 0:
                    # DVE square path (bit-identical: fl(fl(v+b)^2))
                    t_rr = sqp.tile([128, 1536], F32, tag="rr", name=f"rr{g}")
                    nc.vector.tensor_scalar(t_rr[:], pnv[:], t_eb[:], None, OP.add)
                    nc.vector.tensor_tensor(t_sq[:], t_rr[:], t_rr[:], OP.mult)
                else:
                    nc.scalar.activation(t_sq[:], pnv[:], AF.Square, bias=t_eb[:], scale=1.0)
                if g == 0:
                    nc.sync.dma_start(sq0_out.ap(), t_sq[:])
                t_k = kp.tile([128, 1536], F32, tag="k", name=f"k{g}")
                nc.scalar.activation(t_k[:], t_sq[:], AF.Exp, bias=0.0, scale=C_EXP)
                t_p8 = pp.tile([128, 8], F32, tag="p8", name=f"p8{g}")
                kv = t_k[:].rearrange("p (c f) -> p c f", c=8)
                nc.vector.reduce_sum(t_p8[:], kv, axis=AX.X)
                pv = t_p8[:].rearrange("p (a b) -> p a b", a=2)
                nc.vector.reduce_sum(t_stats[:, 2 * g:2 * g + 2], pv, axis=AX.X)

            # ---- entropy stage (bins on partitions, tiles on free) ----
            t_pdf = wk.tile([128, 128], F32, name="t_pdf")
            nc.vector.tensor_scalar(t_pdf[:], t_stats[:], INV_P, None, OP.mult)
            t_sh = wk.tile([128, 128], F32, name="t_sh")
            nc.vector.tensor_scalar(t_sh[:], t_pdf[:], float(C1), None, OP.subtract)
            p_S = pse.tile([2, 256], F32, name="p_S", tag="pS")
            nc.tensor.matmul(p_S[:, 0:128], t_mask[:], t_sh[:], start=True, stop=True)
            t_S = wk.tile([2, 128], F32, name="t_S")
            nc.vector.tensor_scalar(t_S[:], p_S[:, 0:128], C1_64, None, OP.add)
            t_rS = wk.tile([2, 128], F32, name="t_rS")
            nc.vector.reciprocal(t_rS[:], t_S[:])
            t_nS = wk.tile([2, 128], F32, name="t_nS")
            nc.vector.tensor_scalar(t_nS[:], t_S[:], -1.0, None, OP.mult)
            # broadcast rS, nS to [128,128]
            p_b = pse.tile([128, 512], F32, name="p_b", tag="plate")
            nc.tensor.matmul(p_b[:, 0:128], t_maskT[:], t_rS[:], start=True, stop=True)
            nc.tensor.matmul(p_b[:, 128:256], t_maskT[:], t_nS[:], start=True, stop=True)
            # Newton divide q = pdf/S
            t_qq0 = wk.tile([128, 128], F32, name="t_qq0")
            nc.vector.tensor_tensor(t_qq0[:], t_pdf[:], p_b[:, 0:128], OP.mult)
            t_t1 = wk.tile([128, 128], F32, name="t_t1")
            nc.vector.tensor_tensor(t_t1[:], t_qq0[:], p_b[:, 128:256], OP.mult)
            t_rn = wk.tile([128, 128], F32, name="t_rn")
            nc.vector.tensor_tensor(t_rn[:], t_t1[:], t_pdf[:], OP.add)
            t_t2 = wk.tile([128, 128], F32, name="t_t2")
            nc.vector.tensor_tensor(t_t2[:], t_rn[:], p_b[:, 0:128], OP.mult)
            t_q = wk.tile([128, 128], F32, name="t_q")
            nc.vector.tensor_tensor(t_q[:], t_t2[:], t_qq0[:], OP.add)
            nc.vector.tensor_scalar(t_q[:], t_q[:], 1e-19, None, OP.max)
            t_ln = wk.tile([128, 128], F32, name="t_ln")
            nc.scalar.activation(t_ln[:], t_q[:], AF.Ln, bias=0.0, scale=1.0)
            t_pr = wk.tile([128, 128], F32, name="t_pr")
            nc.vector.tensor_tensor(t_pr[:], t_q[:], t_ln[:], OP.mult)
            t_sh2 = wk.tile([128, 128], F32, name="t_sh2")
            nc.vector.tensor_scalar(t_sh2[:], t_pr[:], float(C2), None, OP.add)
            nc.tensor.matmul(p_S[:, 128:256], t_mask[:], t_sh2[:], start=True, stop=True)
            t_e = wk.tile([2, 128], F32, name="t_e")
            nc.vector.tensor_scalar(t_e[:], p_S[:, 128:256], -1.0, C2_64, OP.mult, OP.add)
            nc.sync.dma_start(ent_out.ap(), t_e[:])
            # row r = 2t + hh -> ag_in[0, r]
            ag_ap = bass.AP(ag_in, 0, [[1, 2], [2, 128]])
            nc.sync.dma_start(ag_ap, t_e[:])

            # ---- AllGather entropies within sample pairs ----
            nc.gpsimd.collective_compute(
                "AllGather", OP.bypass,
                replica_groups=[[0, 1], [2, 3], [4, 5], [6, 7]],
                ins=[ag_in.ap()], outs=[ag_out.ap()],
            )

            # ---- rank stage ----
            t_eall = wk.tile([1, L], F32, name="t_eall")
            nc.sync.dma_start(t_eall[:], ag_out.ap())
            # broadcast eall to 128 partitions (f32 matmul, exact)
            p_e = pse.tile([128, 512], F32, name="p_e", tag="plate")
            nc.tensor.matmul(p_e[:], t_ones1[:], t_eall[:], start=True, stop=True)
            t_iota = wk.tile([128, L], I32, name="t_iota")
            nc.gpsimd.iota(t_iota[:], pattern=[[1, L]], base=0, channel_multiplier=0)
            t_iotaf = wk.tile([128, L], F32, name="t_iotaf")
            nc.vector.tensor_copy(t_iotaf[:], t_iota[:])
            t_pidx = wk.tile([128, 1], I32, name="t_pidx")
            nc.gpsimd.iota(t_pidx[:], pattern=[[0, 1]], base=0, channel_multiplier=1)
            t_pidxf = wk.tile([128, 1], F32, name="t_pidxf")
            nc.vector.tensor_copy(t_pidxf[:], t_pidx[:])

            t_hbb = wk.tile([128, 1], F32, name="t_hbb")
            nc.gpsimd.partition_broadcast(t_hbb[:], t_hb[:], channels=128)
            for ci in range(2):
                # e_me[p] = my row (ci*128+p)'s entropy, from ag_in (row order)
                t_eme = wk.tile([128, 1], F32, name=f"t_eme{ci}")
                nc.sync.dma_start(
                    t_eme[:], bass.AP(ag_in, ci * 128, [[1, 128], [1, 1]]))
                # global row index threshold for stable tie-break
                t_thrf = wk.tile([128, 1], F32, name=f"t_thrf{ci}")
                nc.vector.tensor_tensor(t_thrf[:], t_pidxf[:], t_hbb[:], OP.add)
                if ci:
                    nc.vector.tensor_scalar(t_thrf[:], t_thrf[:], 128.0, None, OP.add)
                t_lt = wk.tile([128, L], F32, name=f"t_lt{ci}")
                nc.vector.tensor_scalar(t_lt[:], p_e[:], t_eme[:], None, OP.is_lt)
                t_eq = wk.tile([128, L], F32, name=f"t_eq{ci}")
                nc.vector.tensor_scalar(t_eq[:], p_e[:], t_eme[:], None, OP.is_equal)
                t_il = wk.tile([128, L], F32, name=f"t_il{ci}")
                nc.vector.tensor_scalar(t_il[:], t_iotaf[:], t_thrf[:], None, OP.is_lt)
                t_m1 = wk.tile([128, L], F32, name=f"t_m1{ci}")
                nc.vector.tensor_tensor(t_m1[:], t_eq[:], t_il[:], OP.mult)
                t_dm = wk.tile([128, L], F32, name=f"t_dm{ci}")
                t_rkf = wk.tile([128, 1], F32, name=f"t_rkf{ci}")
                nc.vector.scalar_tensor_tensor(
                    t_dm[:], t_m1[:], 0.0, t_lt[:], op0=OP.add, op1=OP.add,
                    accum_out=t_rkf[:])
                t_rki = wk.tile([128, 1], I32, name=f"t_rki{ci}")
                nc.vector.tensor_copy(t_rki[:], t_rkf[:])
                nc.sync.dma_start(
                    bass.AP(idr_out, ci * 128, [[1, 128], [1, 1]]),
                    t_rki[:])
                t_msk = wk.tile([128, 1], F32, name=f"t_msk{ci}")
                nc.vector.tensor_scalar(t_msk[:], t_rkf[:], 127.5, None, OP.is_ge)
                nc.sync.dma_start(
                    bass.AP(mask_out, ci * 128, [[1, 128], [1, 1]]),
                    t_msk[:])
                # scatter kept x rows
                nc.gpsimd.indirect_dma_start(
                    xm_out.ap(),
                    IndirectOffsetOnAxis(ap=t_rki[:], axis=0),
                    t_x[:, ci * D:(ci + 1) * D],
                    None,
                    bounds_check=LK - 1,
                    oob_is_err=False,
                )

    nc.compile()
    _NC_CACHE["nc"] = nc
    return nc


def kernel(x, img_pat):
    x = np.ascontiguousarray(x, dtype=np.float32)
    img_pat = np.ascontiguousarray(img_pat, dtype=np.float32)
    nc = _build()

    gimg_full = np.ascontiguousarray(
        img_pat.reshape(8, 128, 12288 // 8).transpose(1, 0, 2).reshape(128, 12288))
    in_maps = []
    for c in range(8):
        n, h = c // 2, c % 2
        ip_rows = img_pat[n, h * RPC:(h + 1) * RPC]          # [256, 768]
        x_rows = x[n, h * RPC:(h + 1) * RPC]                 # [256, 1024]
        in_maps.append(dict(
            ipat=np.ascontiguousarray(
                ip_rows.reshape(2, 128, P).transpose(1, 0, 2).reshape(128, 2 * P)),
            xrows=np.ascontiguousarray(
                x_rows.reshape(2, 128, D).transpose(1, 0, 2).reshape(128, 2 * D)),
            hbase=np.array([[h * RPC]], np.float32),
            gimg=gimg_full,
        ))

    kw = {}
    if os.environ.get("KERNEL_TRACE"):
        kw = dict(trace=True, trace_cores=[0])
    res = run_bass_kernel_spmd(nc, in_maps, core_ids=list(range(8)), **kw)
    LAST_RESULT["res"] = res

    x_masked = np.zeros((N, LK, D), np.float32)
    mask = np.zeros((N, L), np.float32)
    idr = np.zeros((N, L), np.int32)
    for c in range(8):
        n, h = c // 2, c % 2
        o = res.results[c]
        x_masked[n] += o["xm_out"]
        mask[n, h * RPC:(h + 1) * RPC] = o["mask_out"][0]
        idr[n, h * RPC:(h + 1) * RPC] = o["idr_out"][0]
    return x_masked, mask, idr


if __name__ == "__main__":
    import reference
    inputs = {k: np.asarray(v) for k, v in reference.setup_inputs().items()}
    out = kernel(**inputs)
    print([o.shape for o in out])


# revision 26
# speedup vs baseline: 1.0987x; 1.0987x over previous
"""TRN2 Bass kernel for nn_MaskingModule (topk entropy-KDE masking).

Self-contained: builds a Bass/Tile kernel, shards batch*seq across 8
NeuronCores (2 cores per batch sample, 256 rows each), runs via
run_bass_kernel_spmd, reassembles full outputs.

Returns (x_masked [4,128,1024] f32, mask [4,512] f32, ids_restore [4,512] i32).

Numerics are engineered to track the f32 reference bit-closely:
- nv = (v - vmin)/R via reciprocal + one Newton step (matches correctly
  rounded divide to <=1ulp)
- broadcast of nv to 128 partitions via exact 3-part bf16 split matmul
  (truncation split: 8+8+8 mantissa bits, products with one-hot exact,
  PSUM f32 reconstruction bit-exact)
- resid^2 via ACT Square with per-partition bins bias (exact)
- kern via ACT Exp (scale folded, ~3.4e-6 rms LUT err)
- 768-sums chunked 4x192 L->R f32 (DVE)
- entropy-stage reductions via exact f32 matmuls with mean-shift trick
"""
import os
import numpy as np

import concourse.bass as bass
import concourse.bacc as bacc
import concourse.mybir as mybir
import concourse.tile as tile
from concourse.bass import IndirectOffsetOnAxis
import concourse.bass_utils as _bu
from concourse.bass_utils import run_bass_kernel_spmd



F32 = mybir.dt.float32
I32 = mybir.dt.int32
BF16 = mybir.dt.bfloat16
AF = mybir.ActivationFunctionType
OP = mybir.AluOpType
AX = mybir.AxisListType

N, L, D, P = 4, 512, 1024, 768
LK = 128              # len_keep
RPC = 256             # rows per core
NB = 64               # bins

# --- constants replicating the reference's f32 arithmetic ---
BINS = np.arange(NB, dtype=np.float32) * (np.float32(1.0) / np.float32(63.0))
S_INV = np.float32(1.0) / np.float32(0.01)                      # fl(1/sigma)
C_EXP = float(np.float32(-0.5 * np.float64(S_INV) ** 2))        # exp scale
INV_P = float(np.float32(1.0) / np.float32(768.0))
C1 = np.float32(0.025)     # pdf mean-shift (64*C1 exact in f32)
C2 = np.float32(0.0586)    # prod mean-shift
C1_64 = float(np.float64(C1) * 64)
C2_64 = float(np.float64(C2) * 64)


def _make_sel6():
    # sel6[s2] [64,128] bf16: rows 6*s2+{0,1,2} (A parts) -> cols 0..63,
    # rows 6*s2+{3,4,5} (B parts) -> cols 64..127. One matmul contracts
    # hi+mid+lo for both rows of a tile.
    import ml_dtypes
    sels = np.zeros((8, 64, 128), np.float32)
    for s2 in range(8):
        sels[s2, 6 * s2:6 * s2 + 3, :64] = 1.0
        sels[s2, 6 * s2 + 3:6 * s2 + 6, 64:] = 1.0
    block = sels.transpose(1, 0, 2).reshape(64, 8 * 128)
    return np.tile(block, (2, 1)).astype(ml_dtypes.bfloat16)   # [128, 1024]


def _make_masks():
    # maskf [128, 2] f32: rows<64 -> col0, rows>=64 -> col1 (entropy-stage sums)
    mk = np.zeros((128, 2), np.float32)
    mk[:64, 0] = 1.0
    mk[64:, 1] = 1.0
    # maskT [2, 128] f32 for broadcasting [2,x]->[128,x]
    return mk, mk.T.copy()


_NC_CACHE = {}
LAST_RESULT = {}


def _build():
    if "nc" in _NC_CACHE:
        return _NC_CACHE["nc"]
    nc = bacc.Bacc("TRN2", target_bir_lowering=False, debug=False, num_devices=8)

    ipat = nc.dram_tensor("ipat", [128, 2 * P], F32, kind="ExternalInput")
    xrows = nc.dram_tensor("xrows", [128, 2 * D], F32, kind="ExternalInput")
    hbase = nc.dram_tensor("hbase", [1, 1], F32, kind="ExternalInput")  # h*256
    gimg = nc.dram_tensor("gimg", [128, 12288], F32, kind="ExternalInput")

    sel_d = nc.inline_tensor(_make_sel6(), name="sel_d")
    ebias_d = nc.inline_tensor(
        BINS[np.arange(128) % NB].reshape(128, 1).astype(np.float32), name="ebias_d")
    mk, mkT = _make_masks()
    mask_d = nc.inline_tensor(mk, name="mask_d")
    maskT_d = nc.inline_tensor(mkT, name="maskT_d")
    ones1_d = nc.inline_tensor(np.ones((1, 128), np.float32), name="ones1_d")

    xm_out = nc.dram_tensor("xm_out", [128, D], F32, kind="ExternalOutput")
    idr_out = nc.dram_tensor("idr_out", [1, RPC], I32, kind="ExternalOutput")
    mask_out = nc.dram_tensor("mask_out", [1, RPC], F32, kind="ExternalOutput")
    ent_out = nc.dram_tensor("ent_out", [2, 128], F32, kind="ExternalOutput")

    ag_in = nc.dram_tensor("ag_in", [1, RPC], F32, kind="Internal")
    ag_out = nc.dram_tensor("ag_out", [1, L], F32, kind="Internal")

    with tile.TileContext(nc) as tc:
        with (
            tc.tile_pool(name="cst", bufs=1) as cst,
            tc.tile_pool(name="work", bufs=1) as wk,
            tc.tile_pool(name="sq", bufs=2) as sqp,
            tc.tile_pool(name="kern", bufs=2) as kp,
            tc.tile_pool(name="p4", bufs=2) as pp,
            tc.tile_pool(name="ps", bufs=2, space="PSUM") as ps,
            tc.tile_pool(name="pse", bufs=1, space="PSUM") as pse,
        ):
            # ---- load inputs & constants ----
            t_ip = cst.tile([128, 2 * P], F32, name="t_ip")
            nc.sync.dma_start(t_ip[:], ipat.ap())
            t_sel = cst.tile([128, 1024], BF16, name="t_sel")
            nc.sync.dma_start(t_sel[:], sel_d.ap())
            t_eb = cst.tile([128, 1], F32, name="t_eb")
            nc.sync.dma_start(t_eb[:], ebias_d.ap())
            t_mask = cst.tile([128, 2], F32, name="t_mask")
            nc.sync.dma_start(t_mask[:], mask_d.ap())
            t_maskT = cst.tile([2, 128], F32, name="t_maskT")
            nc.sync.dma_start(t_maskT[:], maskT_d.ap())
            t_ones1 = cst.tile([1, 128], F32, name="t_ones1")
            nc.sync.dma_start(t_ones1[:], ones1_d.ap())
            t_x = cst.tile([128, 2 * D], F32, name="t_x")
            nc.sync.dma_start(t_x[:], xrows.ap())
            t_hb = cst.tile([1, 1], F32, name="t_hb")
            nc.sync.dma_start(t_hb[:], hbase.ap())

            # ---- global min/max: every core reduces ALL of img_pat ----
            # (f32 min/max is exact in any order; ~25us DMA-pipelined beats
            # the ~55us AllReduce round trip)
            t_pm = wk.tile([128, 16], F32, name="t_pm")
            for k in range(8):
                t_gik = wk.tile([128, 1536], F32, name=f"t_gi{k}", tag="gik", bufs=3)
                nc.sync.dma_start(t_gik[:], gimg.ap()[:, 1536 * k:1536 * (k + 1)])
                nc.vector.tensor_reduce(t_pm[:, k:k + 1], t_gik[:], axis=AX.X, op=OP.max)
                nc.vector.tensor_reduce(t_pm[:, 8 + k:9 + k], t_gik[:], axis=AX.X, op=OP.min)
            t_mm2 = wk.tile([128, 2], F32, name="t_mm2")
            nc.vector.tensor_reduce(t_mm2[:, 0:1], t_pm[:, 0:8], axis=AX.X, op=OP.max)
            t_rmin = wk.tile([128, 1], F32, name="t_rmin")
            nc.vector.tensor_reduce(t_rmin[:], t_pm[:, 8:16], axis=AX.X, op=OP.min)
            nc.vector.tensor_scalar(t_mm2[:, 1:2], t_rmin[:], -1.0, None, OP.mult)
            import concourse.bass_isa as bass_isa
            t_mmr = wk.tile([128, 2], F32, name="t_mmr")
            nc.gpsimd.partition_all_reduce(
                t_mmr[:], t_mm2[:], channels=128, reduce_op=bass_isa.ReduceOp.max)

            # derive [negmin, R, recipR] on partition 0, broadcast to 128
            t_pack = wk.tile([1, 3], F32, name="t_pack")
            nc.vector.tensor_copy(t_pack[:, 0:1], t_mmr[0:1, 1:2])      # negmin
            nc.vector.tensor_tensor(t_pack[:, 1:2], t_mmr[0:1, 0:1], t_mmr[0:1, 1:2], OP.add)  # R
            nc.vector.reciprocal(t_pack[:, 2:3], t_pack[:, 1:2])        # r
            t_cb = wk.tile([128, 3], F32, name="t_cb")
            nc.gpsimd.partition_broadcast(t_cb[:], t_pack[:], channels=128)

            # ---- Newton nv (negated): nvneg = (rem*r) - q0 ----
            t_d = wk.tile([128, 2 * P], F32, name="t_d")
            nc.vector.tensor_scalar(t_d[:], t_ip[:], t_cb[:, 0:1], None, OP.add)
            t_q0 = wk.tile([128, 2 * P], F32, name="t_q0")
            nc.vector.tensor_scalar(t_q0[:], t_d[:], t_cb[:, 2:3], None, OP.mult)
            t_rem = wk.tile([128, 2 * P], F32, name="t_rem")
            nc.vector.scalar_tensor_tensor(
                t_rem[:], t_q0[:], t_cb[:, 1:2], t_d[:], op0=OP.mult, op1=OP.subtract)
            t_nvn = wk.tile([128, 2 * P], F32, name="t_nvn")
            nc.vector.scalar_tensor_tensor(
                t_nvn[:], t_rem[:], t_cb[:, 2:3], t_q0[:], op0=OP.mult, op1=OP.subtract)

            # ---- exact 3-part truncation split to bf16 ----
            t_hi32 = wk.tile([128, 2 * P], F32, name="t_hi32")
            nc.vector.tensor_scalar(
                t_hi32[:].bitcast(mybir.dt.uint32), t_nvn[:].bitcast(mybir.dt.uint32),
                0xFFFF0000, None, OP.bitwise_and)
            t_r1 = wk.tile([128, 2 * P], F32, name="t_r1")
            nc.vector.tensor_tensor(t_r1[:], t_nvn[:], t_hi32[:], OP.subtract)
            t_mid32 = wk.tile([128, 2 * P], F32, name="t_mid32")
            nc.vector.tensor_scalar(
                t_mid32[:].bitcast(mybir.dt.uint32), t_r1[:].bitcast(mybir.dt.uint32),
                0xFFFF0000, None, OP.bitwise_and)
            t_hi = wk.tile([128, 2 * P], BF16, name="t_hi")
            t_mid = wk.tile([128, 2 * P], BF16, name="t_mid")
            t_lo = wk.tile([128, 2 * P], BF16, name="t_lo")
            nc.vector.tensor_copy(t_hi[:], t_hi32[:])
            nc.vector.tensor_copy(t_mid[:], t_mid32[:])
            nc.vector.tensor_tensor(t_lo[:], t_r1[:], t_mid32[:], OP.subtract)

            # ---- interleave parts by partition: window (parity, blk) holds
            # 8 tiles; tile slot s2 occupies partitions 6*s2..6*s2+5 =
            # (hiA,midA,loA,hiB,midB,loB); one K=64 matmul sums all parts.
            t_pkb = []
            for blk in range(8):
                tb = cst.tile([128, 768], BF16, name=f"t_pkb{blk}", tag=f"pkb{blk}")
                # partitions 48-63/112-127 are unused by the one-hot lhsT but
                # must be finite: 0*NaN = NaN would poison the PSUM.
                nc.gpsimd.memset(tb[:], 0.0)
                t_pkb.append(tb)
            parts3 = (t_hi, t_mid, t_lo)
            for ci in range(2):
                for j in range(4):
                    blk = ci * 4 + j
                    for parity in range(2):
                        for kpart in range(3):
                            sp = 16 * (parity + 2 * j)
                            b0 = 64 * parity + kpart
                            nc.sync.dma_start(
                                t_pkb[blk][b0:b0 + 46:3, 0:768],
                                parts3[kpart][sp:sp + 16, ci * P:ci * P + 768])

            # ---- main loop: 32 groups x 2 tiles x 2 rows ----
            t_stats = cst.tile([128, 128], F32, name="t_stats")
            for g in range(64):
                # tiles 2g (rows 4g,4g+1) and 2g+1 (rows 4g+2,4g+3)
                pnv = ps.tile([128, 1536], F32, tag="pnv", name=f"pnv{g}")
                for half in range(2):           # A: psum cols 0:768, B: 768:1536
                    t = 2 * g + half
                    ci = t // 64
                    qq = (t % 64) // 8
                    s2 = t % 8
                    parity = qq % 2
                    blk = ci * 4 + qq // 2
                    lhs_ap = t_sel[64 * parity:64 * parity + 64,
                                   s2 * 128:(s2 + 1) * 128]
                    rb = t_pkb[blk]
                    base = half * 768
                    # regions must not cross PSUM bank boundaries (512 f32)
                    regions = ((0, 512), (512, 768)) if half == 0 else ((0, 256), (256, 768))
                    for (c0, c1) in regions:
                        nc.tensor.matmul(
                            pnv[:, base + c0:base + c1], lhs_ap,
                            rb[64 * parity:64 * parity + 64, c0:c1],
                            start=True, stop=True,
                        )
                t_sq = sqp.tile([128, 1536], F32, tag="sq", name=f"sq{g}")
                if g % 4 == 0:
                    # DVE square path (bit-identical: fl(fl(v+b)^2))
                    t_rr = sqp.tile([128, 1536], F32, tag="rr", name=f"rr{g}")
                    nc.vector.tensor_scalar(t_rr[:], pnv[:], t_eb[:], None, OP.add)
                    nc.vector.tensor_tensor(t_sq[:], t_rr[:], t_rr[:], OP.mult)
                else:
                    nc.scalar.activation(t_sq[:], pnv[:], AF.Square, bias=t_eb[:], scale=1.0)
                t_k = kp.tile([128, 1536], F32, tag="k", name=f"k{g}")
                nc.scalar.activation(t_k[:], t_sq[:], AF.Exp, bias=0.0, scale=C_EXP)
                t_p8 = pp.tile([128, 8], F32, tag="p8", name=f"p8{g}")
                kv = t_k[:].rearrange("p (c f) -> p c f", c=8)
                nc.vector.reduce_sum(t_p8[:], kv, axis=AX.X)
                pv = t_p8[:].rearrange("p (a b) -> p a b", a=2)
                nc.vector.reduce_sum(t_stats[:, 2 * g:2 * g + 2], pv, axis=AX.X)

            # ---- entropy stage (bins on partitions, tiles on free) ----
            t_pdf = wk.tile([128, 128], F32, name="t_pdf")
            nc.vector.tensor_scalar(t_pdf[:], t_stats[:], INV_P, None, OP.mult)
            t_sh = wk.tile([128, 128], F32, name="t_sh")
            nc.vector.tensor_scalar(t_sh[:], t_pdf[:], float(C1), None, OP.subtract)
            p_S = pse.tile([2, 256], F32, name="p_S", tag="pS")
            nc.tensor.matmul(p_S[:, 0:128], t_mask[:], t_sh[:], start=True, stop=True)
            t_S = wk.tile([2, 128], F32, name="t_S")
            nc.vector.tensor_scalar(t_S[:], p_S[:, 0:128], C1_64, None, OP.add)
            t_rS = wk.tile([2, 128], F32, name="t_rS")
            nc.vector.reciprocal(t_rS[:], t_S[:])
            t_nS = wk.tile([2, 128], F32, name="t_nS")
            nc.vector.tensor_scalar(t_nS[:], t_S[:], -1.0, None, OP.mult)
            # broadcast rS, nS to [128,128]
            p_b = pse.tile([128, 512], F32, name="p_b", tag="plate")
            nc.tensor.matmul(p_b[:, 0:128], t_maskT[:], t_rS[:], start=True, stop=True)
            nc.tensor.matmul(p_b[:, 128:256], t_maskT[:], t_nS[:], start=True, stop=True)
            # Newton divide q = pdf/S
            t_qq0 = wk.tile([128, 128], F32, name="t_qq0")
            nc.vector.tensor_tensor(t_qq0[:], t_pdf[:], p_b[:, 0:128], OP.mult)
            t_t1 = wk.tile([128, 128], F32, name="t_t1")
            nc.vector.tensor_tensor(t_t1[:], t_qq0[:], p_b[:, 128:256], OP.mult)
            t_rn = wk.tile([128, 128], F32, name="t_rn")
            nc.vector.tensor_tensor(t_rn[:], t_t1[:], t_pdf[:], OP.add)
            t_t2 = wk.tile([128, 128], F32, name="t_t2")
            nc.vector.tensor_tensor(t_t2[:], t_rn[:], p_b[:, 0:128], OP.mult)
            t_q = wk.tile([128, 128], F32, name="t_q")
            nc.vector.tensor_tensor(t_q[:], t_t2[:], t_qq0[:], OP.add)
            nc.vector.tensor_scalar(t_q[:], t_q[:], 1e-19, None, OP.max)
            t_ln = wk.tile([128, 128], F32, name="t_ln")
            nc.scalar.activation(t_ln[:], t_q[:], AF.Ln, bias=0.0, scale=1.0)
            t_pr = wk.tile([128, 128], F32, name="t_pr")
            nc.vector.tensor_tensor(t_pr[:], t_q[:], t_ln[:], OP.mult)
            t_sh2 = wk.tile([128, 128], F32, name="t_sh2")
            nc.vector.tensor_scalar(t_sh2[:], t_pr[:], float(C2), None, OP.add)
            nc.tensor.matmul(p_S[:, 128:256], t_mask[:], t_sh2[:], start=True, stop=True)
            t_e = wk.tile([2, 128], F32, name="t_e")
            nc.vector.tensor_scalar(t_e[:], p_S[:, 128:256], -1.0, C2_64, OP.mult, OP.add)
            nc.sync.dma_start(ent_out.ap(), t_e[:])
            # row r = 2t + hh -> ag_in[0, r]
            ag_ap = bass.AP(ag_in, 0, [[1, 2], [2, 128]])
            nc.sync.dma_start(ag_ap, t_e[:])

            # ---- AllGather entropies within sample pairs ----
            nc.gpsimd.collective_compute(
                "AllGather", OP.bypass,
                replica_groups=[[0, 1], [2, 3], [4, 5], [6, 7]],
                ins=[ag_in.ap()], outs=[ag_out.ap()],
            )

            # ---- rank stage ----
            t_eall = wk.tile([1, L], F32, name="t_eall")
            nc.sync.dma_start(t_eall[:], ag_out.ap())
            # broadcast eall to 128 partitions (f32 matmul, exact)
            p_e = pse.tile([128, 512], F32, name="p_e", tag="plate")
            nc.tensor.matmul(p_e[:], t_ones1[:], t_eall[:], start=True, stop=True)
            t_iota = wk.tile([128, L], I32, name="t_iota")
            nc.gpsimd.iota(t_iota[:], pattern=[[1, L]], base=0, channel_multiplier=0)
            t_iotaf = wk.tile([128, L], F32, name="t_iotaf")
            nc.vector.tensor_copy(t_iotaf[:], t_iota[:])
            t_pidx = wk.tile([128, 1], I32, name="t_pidx")
            nc.gpsimd.iota(t_pidx[:], pattern=[[0, 1]], base=0, channel_multiplier=1)
            t_pidxf = wk.tile([128, 1], F32, name="t_pidxf")
            nc.vector.tensor_copy(t_pidxf[:], t_pidx[:])

            t_hbb = wk.tile([128, 1], F32, name="t_hbb")
            nc.gpsimd.partition_broadcast(t_hbb[:], t_hb[:], channels=128)
            for ci in range(2):
                # e_me[p] = my row (ci*128+p)'s entropy, from ag_in (row order)
                t_eme = wk.tile([128, 1], F32, name=f"t_eme{ci}")
                nc.sync.dma_start(
                    t_eme[:], bass.AP(ag_in, ci * 128, [[1, 128], [1, 1]]))
                # global row index threshold for stable tie-break
                t_thrf = wk.tile([128, 1], F32, name=f"t_thrf{ci}")
                nc.vector.tensor_tensor(t_thrf[:], t_pidxf[:], t_hbb[:], OP.add)
                if ci:
                    nc.vector.tensor_scalar(t_thrf[:], t_thrf[:], 128.0, None, OP.add)
                t_lt = wk.tile([128, L], F32, name=f"t_lt{ci}")
                nc.vector.tensor_scalar(t_lt[:], p_e[:], t_eme[:], None, OP.is_lt)
                t_eq = wk.tile([128, L], F32, name=f"t_eq{ci}")
                nc.vector.tensor_scalar(t_eq[:], p_e[:], t_eme[:], None, OP.is_equal)
                t_il = wk.tile([128, L], F32, name=f"t_il{ci}")
                nc.vector.tensor_scalar(t_il[:], t_iotaf[:], t_thrf[:], None, OP.is_lt)
                t_m1 = wk.tile([128, L], F32, name=f"t_m1{ci}")
                nc.vector.tensor_tensor(t_m1[:], t_eq[:], t_il[:], OP.mult)
                t_dm = wk.tile([128, L], F32, name=f"t_dm{ci}")
                t_rkf = wk.tile([128, 1], F32, name=f"t_rkf{ci}")
                nc.vector.scalar_tensor_tensor(
                    t_dm[:], t_m1[:], 0.0, t_lt[:], op0=OP.add, op1=OP.add,
                    accum_out=t_rkf[:])
                t_rki = wk.tile([128, 1], I32, name=f"t_rki{ci}")
                nc.vector.tensor_copy(t_rki[:], t_rkf[:])
                nc.sync.dma_start(
                    bass.AP(idr_out, ci * 128, [[1, 128], [1, 1]]),
                    t_rki[:])
                t_msk = wk.tile([128, 1], F32, name=f"t_msk{ci}")
                nc.vector.tensor_scalar(t_msk[:], t_rkf[:], 127.5, None, OP.is_ge)
                nc.sync.dma_start(
                    bass.AP(mask_out, ci * 128, [[1, 128], [1, 1]]),
                    t_msk[:])
                # scatter kept x rows
                nc.gpsimd.indirect_dma_start(
                    xm_out.ap(),
                    IndirectOffsetOnAxis(ap=t_rki[:], axis=0),
                    t_x[:, ci * D:(ci + 1) * D],
                    None,
                    bounds_check=LK - 1,
                    oob_is_err=False,
                )

    nc.compile()
    _NC_CACHE["nc"] = nc
    return nc


def kernel(x, img_pat):
    x = np.ascontiguousarray(x, dtype=np.float32)
    img_pat = np.ascontiguousarray(img_pat, dtype=np.float32)
    nc = _build()

    gimg_full = np.ascontiguousarray(
        img_pat.reshape(8, 128, 12288 // 8).transpose(1, 0, 2).reshape(128, 12288))
    in_maps = []
    for c in range(8):
        n, h = c // 2, c % 2
        ip_rows = img_pat[n, h * RPC:(h + 1) * RPC]          # [256, 768]
        x_rows = x[n, h * RPC:(h + 1) * RPC]                 # [256, 1024]
        in_maps.append(dict(
            ipat=np.ascontiguousarray(
                ip_rows.reshape(2, 128, P).transpose(1, 0, 2).reshape(128, 2 * P)),
            xrows=np.ascontiguousarray(
                x_rows.reshape(2, 128, D).transpose(1, 0, 2).reshape(128, 2 * D)),
            hbase=np.array([[h * RPC]], np.float32),
            gimg=gimg_full,
        ))

    kw = {}
    if os.environ.get("KERNEL_TRACE"):
        kw = dict(trace=True, trace_cores=[0])
    res = run_bass_kernel_spmd(nc, in_maps, core_ids=list(range(8)), **kw)
    LAST_RESULT["res"] = res

    x_masked = np.zeros((N, LK, D), np.float32)
    mask = np.zeros((N, L), np.float32)
    idr = np.zeros((N, L), np.int32)
    for c in range(8):
        n, h = c // 2, c % 2
        o = res.results[c]
        x_masked[n] += o["xm_out"]
        mask[n, h * RPC:(h + 1) * RPC] = o["mask_out"][0]
        idr[n, h * RPC:(h + 1) * RPC] = o["idr_out"][0]
    return x_masked, mask, idr


if __name__ == "__main__":
    import reference
    inputs = {k: np.asarray(v) for k, v in reference.setup_inputs().items()}
    out = kernel(**inputs)
    print([o.shape for o in out])


# revision 27
# speedup vs baseline: 1.2250x; 1.1149x over previous
"""TRN2 Bass kernel for nn_MaskingModule (topk entropy-KDE masking).

Self-contained: builds a Bass/Tile kernel, shards batch*seq across 8
NeuronCores (2 cores per batch sample, 256 rows each), runs via
run_bass_kernel_spmd, reassembles full outputs.

Returns (x_masked [4,128,1024] f32, mask [4,512] f32, ids_restore [4,512] i32).

Numerics are engineered to track the f32 reference bit-closely:
- nv = (v - vmin)/R via reciprocal + one Newton step (matches correctly
  rounded divide to <=1ulp)
- broadcast of nv to 128 partitions via exact 3-part bf16 split matmul
  (truncation split: 8+8+8 mantissa bits, products with one-hot exact,
  PSUM f32 reconstruction bit-exact)
- resid^2 via ACT Square with per-partition bins bias (exact)
- kern via ACT Exp (scale folded, ~3.4e-6 rms LUT err)
- 768-sums chunked 4x192 L->R f32 (DVE)
- entropy-stage reductions via exact f32 matmuls with mean-shift trick
"""
import os
import numpy as np

import concourse.bass as bass
import concourse.bacc as bacc
import concourse.mybir as mybir
import concourse.tile as tile
from concourse.bass import IndirectOffsetOnAxis
import concourse.bass_utils as _bu
from concourse.bass_utils import run_bass_kernel_spmd



F32 = mybir.dt.float32
I32 = mybir.dt.int32
BF16 = mybir.dt.bfloat16
AF = mybir.ActivationFunctionType
OP = mybir.AluOpType
AX = mybir.AxisListType

N, L, D, P = 4, 512, 1024, 768
LK = 128              # len_keep
RPC = 256             # rows per core
NB = 64               # bins

# --- constants replicating the reference's f32 arithmetic ---
BINS = np.arange(NB, dtype=np.float32) * (np.float32(1.0) / np.float32(63.0))
S_INV = np.float32(1.0) / np.float32(0.01)                      # fl(1/sigma)
C_EXP = float(np.float32(-0.5 * np.float64(S_INV) ** 2))        # exp scale
INV_P = float(np.float32(1.0) / np.float32(768.0))
C1 = np.float32(0.025)     # pdf mean-shift (64*C1 exact in f32)
C2 = np.float32(0.0586)    # prod mean-shift
C1_64 = float(np.float64(C1) * 64)
C2_64 = float(np.float64(C2) * 64)


def _make_sel6():
    # sel6[s2] [64,128] bf16: rows 6*s2+{0,1,2} (A parts) -> cols 0..63,
    # rows 6*s2+{3,4,5} (B parts) -> cols 64..127. One matmul contracts
    # hi+mid+lo for both rows of a tile.
    import ml_dtypes
    sels = np.zeros((8, 64, 128), np.float32)
    for s2 in range(8):
        sels[s2, 6 * s2:6 * s2 + 3, :64] = 1.0
        sels[s2, 6 * s2 + 3:6 * s2 + 6, 64:] = 1.0
    block = sels.transpose(1, 0, 2).reshape(64, 8 * 128)
    return np.tile(block, (2, 1)).astype(ml_dtypes.bfloat16)   # [128, 1024]


def _make_masks():
    # maskf [128, 2] f32: rows<64 -> col0, rows>=64 -> col1 (entropy-stage sums)
    mk = np.zeros((128, 2), np.float32)
    mk[:64, 0] = 1.0
    mk[64:, 1] = 1.0
    # maskT [2, 128] f32 for broadcasting [2,x]->[128,x]
    return mk, mk.T.copy()


_NC_CACHE = {}
LAST_RESULT = {}


def _build():
    if "nc" in _NC_CACHE:
        return _NC_CACHE["nc"]
    nc = bacc.Bacc("TRN2", target_bir_lowering=False, debug=False, num_devices=8)

    ipat = nc.dram_tensor("ipat", [128, 2 * P], F32, kind="ExternalInput")
    xrows = nc.dram_tensor("xrows", [128, 2 * D], F32, kind="ExternalInput")
    hbase = nc.dram_tensor("hbase", [1, 1], F32, kind="ExternalInput")  # h*256
    gimg = nc.dram_tensor("gimg", [128, 12288], F32, kind="ExternalInput")

    sel_d = nc.inline_tensor(_make_sel6(), name="sel_d")
    ebias_d = nc.inline_tensor(
        BINS[np.arange(128) % NB].reshape(128, 1).astype(np.float32), name="ebias_d")
    mk, mkT = _make_masks()
    mask_d = nc.inline_tensor(mk, name="mask_d")
    maskT_d = nc.inline_tensor(mkT, name="maskT_d")
    ones1_d = nc.inline_tensor(np.ones((1, 128), np.float32), name="ones1_d")

    xm_out = nc.dram_tensor("xm_out", [128, D], F32, kind="ExternalOutput")
    idr_out = nc.dram_tensor("idr_out", [1, RPC], I32, kind="ExternalOutput")
    mask_out = nc.dram_tensor("mask_out", [1, RPC], F32, kind="ExternalOutput")
    ent_out = nc.dram_tensor("ent_out", [2, 128], F32, kind="ExternalOutput")

    ag_in = nc.dram_tensor("ag_in", [1, RPC], F32, kind="Internal")
    ag_out = nc.dram_tensor("ag_out", [1, L], F32, kind="Internal")

    with tile.TileContext(nc) as tc:
        with (
            tc.tile_pool(name="cst", bufs=1) as cst,
            tc.tile_pool(name="work", bufs=1) as wk,
            tc.tile_pool(name="sq", bufs=2) as sqp,
            tc.tile_pool(name="kern", bufs=2) as kp,
            tc.tile_pool(name="p4", bufs=2) as pp,
            tc.tile_pool(name="ps", bufs=2, space="PSUM") as ps,
            tc.tile_pool(name="pse", bufs=1, space="PSUM") as pse,
        ):
            # ---- load inputs & constants ----
            t_ip = cst.tile([128, 2 * P], F32, name="t_ip")
            nc.sync.dma_start(t_ip[:], ipat.ap())
            t_sel = cst.tile([128, 1024], BF16, name="t_sel")
            nc.sync.dma_start(t_sel[:], sel_d.ap())
            t_eb = cst.tile([128, 1], F32, name="t_eb")
            nc.sync.dma_start(t_eb[:], ebias_d.ap())
            t_mask = cst.tile([128, 2], F32, name="t_mask")
            nc.sync.dma_start(t_mask[:], mask_d.ap())
            t_maskT = cst.tile([2, 128], F32, name="t_maskT")
            nc.sync.dma_start(t_maskT[:], maskT_d.ap())
            t_ones1 = cst.tile([1, 128], F32, name="t_ones1")
            nc.sync.dma_start(t_ones1[:], ones1_d.ap())
            t_x = cst.tile([128, 2 * D], F32, name="t_x")
            nc.sync.dma_start(t_x[:], xrows.ap())
            t_hb = cst.tile([1, 1], F32, name="t_hb")
            nc.sync.dma_start(t_hb[:], hbase.ap())

            # ---- global min/max: every core reduces ALL of img_pat ----
            # (f32 min/max is exact in any order; ~25us DMA-pipelined beats
            # the ~55us AllReduce round trip)
            t_pm = wk.tile([128, 16], F32, name="t_pm")
            for k in range(8):
                t_gik = wk.tile([128, 1536], F32, name=f"t_gi{k}", tag="gik", bufs=3)
                nc.sync.dma_start(t_gik[:], gimg.ap()[:, 1536 * k:1536 * (k + 1)])
                nc.vector.tensor_reduce(t_pm[:, k:k + 1], t_gik[:], axis=AX.X, op=OP.max)
                nc.vector.tensor_reduce(t_pm[:, 8 + k:9 + k], t_gik[:], axis=AX.X, op=OP.min)
            t_mm2 = wk.tile([128, 2], F32, name="t_mm2")
            nc.vector.tensor_reduce(t_mm2[:, 0:1], t_pm[:, 0:8], axis=AX.X, op=OP.max)
            t_rmin = wk.tile([128, 1], F32, name="t_rmin")
            nc.vector.tensor_reduce(t_rmin[:], t_pm[:, 8:16], axis=AX.X, op=OP.min)
            nc.vector.tensor_scalar(t_mm2[:, 1:2], t_rmin[:], -1.0, None, OP.mult)
            import concourse.bass_isa as bass_isa
            t_mmr = wk.tile([128, 2], F32, name="t_mmr")
            nc.gpsimd.partition_all_reduce(
                t_mmr[:], t_mm2[:], channels=128, reduce_op=bass_isa.ReduceOp.max)

            # derive [negmin, R, recipR] on partition 0, broadcast to 128
            t_pack = wk.tile([1, 3], F32, name="t_pack")
            nc.vector.tensor_copy(t_pack[:, 0:1], t_mmr[0:1, 1:2])      # negmin
            nc.vector.tensor_tensor(t_pack[:, 1:2], t_mmr[0:1, 0:1], t_mmr[0:1, 1:2], OP.add)  # R
            nc.vector.reciprocal(t_pack[:, 2:3], t_pack[:, 1:2])        # r
            t_cb = wk.tile([128, 3], F32, name="t_cb")
            nc.gpsimd.partition_broadcast(t_cb[:], t_pack[:], channels=128)

            # ---- Newton nv (negated): nvneg = (rem*r) - q0 ----
            t_d = wk.tile([128, 2 * P], F32, name="t_d")
            nc.vector.tensor_scalar(t_d[:], t_ip[:], t_cb[:, 0:1], None, OP.add)
            t_q0 = wk.tile([128, 2 * P], F32, name="t_q0")
            nc.vector.tensor_scalar(t_q0[:], t_d[:], t_cb[:, 2:3], None, OP.mult)
            t_rem = wk.tile([128, 2 * P], F32, name="t_rem")
            nc.vector.scalar_tensor_tensor(
                t_rem[:], t_q0[:], t_cb[:, 1:2], t_d[:], op0=OP.mult, op1=OP.subtract)
            t_nvn = wk.tile([128, 2 * P], F32, name="t_nvn")
            nc.vector.scalar_tensor_tensor(
                t_nvn[:], t_rem[:], t_cb[:, 2:3], t_q0[:], op0=OP.mult, op1=OP.subtract)

            # ---- exact 3-part truncation split to bf16 ----
            t_hi32 = wk.tile([128, 2 * P], F32, name="t_hi32")
            nc.vector.tensor_scalar(
                t_hi32[:].bitcast(mybir.dt.uint32), t_nvn[:].bitcast(mybir.dt.uint32),
                0xFFFF0000, None, OP.bitwise_and)
            t_r1 = wk.tile([128, 2 * P], F32, name="t_r1")
            nc.vector.tensor_tensor(t_r1[:], t_nvn[:], t_hi32[:], OP.subtract)
            t_mid32 = wk.tile([128, 2 * P], F32, name="t_mid32")
            nc.vector.tensor_scalar(
                t_mid32[:].bitcast(mybir.dt.uint32), t_r1[:].bitcast(mybir.dt.uint32),
                0xFFFF0000, None, OP.bitwise_and)
            t_hi = wk.tile([128, 2 * P], BF16, name="t_hi")
            t_mid = wk.tile([128, 2 * P], BF16, name="t_mid")
            t_lo = wk.tile([128, 2 * P], BF16, name="t_lo")
            nc.vector.tensor_copy(t_hi[:], t_hi32[:])
            nc.vector.tensor_copy(t_mid[:], t_mid32[:])
            nc.vector.tensor_tensor(t_lo[:], t_r1[:], t_mid32[:], OP.subtract)

            # ---- interleave parts by partition: window (parity, blk) holds
            # 8 tiles; tile slot s2 occupies partitions 6*s2..6*s2+5 =
            # (hiA,midA,loA,hiB,midB,loB); one K=64 matmul sums all parts.
            t_pkb = []
            for blk in range(8):
                tb = cst.tile([128, 768], BF16, name=f"t_pkb{blk}", tag=f"pkb{blk}")
                # partitions 48-63/112-127 are unused by the one-hot lhsT but
                # must be finite: 0*NaN = NaN would poison the PSUM.
                nc.gpsimd.memset(tb[:], 0.0)
                t_pkb.append(tb)
            parts3 = (t_hi, t_mid, t_lo)
            for ci in range(2):
                for j in range(4):
                    blk = ci * 4 + j
                    for parity in range(2):
                        for kpart in range(3):
                            sp = 16 * (parity + 2 * j)
                            b0 = 64 * parity + kpart
                            nc.sync.dma_start(
                                t_pkb[blk][b0:b0 + 46:3, 0:768],
                                parts3[kpart][sp:sp + 16, ci * P:ci * P + 768])

            # ---- main loop: 32 groups x 2 tiles x 2 rows ----
            t_stats = cst.tile([128, 128], F32, name="t_stats")
            for g in range(64):
                # tiles 2g (rows 4g,4g+1) and 2g+1 (rows 4g+2,4g+3)
                pnv = ps.tile([128, 1536], F32, tag="pnv", name=f"pnv{g}")
                for half in range(2):           # A: psum cols 0:768, B: 768:1536
                    t = 2 * g + half
                    ci = t // 64
                    qq = (t % 64) // 8
                    s2 = t % 8
                    parity = qq % 2
                    blk = ci * 4 + qq // 2
                    lhs_ap = t_sel[64 * parity:64 * parity + 64,
                                   s2 * 128:(s2 + 1) * 128]
                    rb = t_pkb[blk]
                    base = half * 768
                    # regions must not cross PSUM bank boundaries (512 f32)
                    regions = ((0, 512), (512, 768)) if half == 0 else ((0, 256), (256, 768))
                    for (c0, c1) in regions:
                        nc.tensor.matmul(
                            pnv[:, base + c0:base + c1], lhs_ap,
                            rb[64 * parity:64 * parity + 64, c0:c1],
                            start=True, stop=True,
                        )
                t_sq = sqp.tile([128, 1536], F32, tag="sq", name=f"sq{g}")
                if g % 8 == 0:
                    # DVE square path (bit-identical: fl(fl(v+b)^2))
                    t_rr = sqp.tile([128, 1536], F32, tag="rr", name=f"rr{g}")
                    nc.vector.tensor_scalar(t_rr[:], pnv[:], t_eb[:], None, OP.add)
                    nc.vector.tensor_tensor(t_sq[:], t_rr[:], t_rr[:], OP.mult)
                else:
                    nc.scalar.activation(t_sq[:], pnv[:], AF.Square, bias=t_eb[:], scale=1.0)
                t_k = kp.tile([128, 1536], F32, tag="k", name=f"k{g}")
                nc.scalar.activation(t_k[:], t_sq[:], AF.Exp, bias=0.0, scale=C_EXP)
                t_p8 = pp.tile([128, 8], F32, tag="p8", name=f"p8{g}")
                kv = t_k[:].rearrange("p (c f) -> p c f", c=8)
                nc.vector.reduce_sum(t_p8[:], kv, axis=AX.X)
                pv = t_p8[:].rearrange("p (a b) -> p a b", a=2)
                nc.vector.reduce_sum(t_stats[:, 2 * g:2 * g + 2], pv, axis=AX.X)

            # ---- entropy stage (bins on partitions, tiles on free) ----
            t_pdf = wk.tile([128, 128], F32, name="t_pdf")
            nc.vector.tensor_scalar(t_pdf[:], t_stats[:], INV_P, None, OP.mult)
            t_sh = wk.tile([128, 128], F32, name="t_sh")
            nc.vector.tensor_scalar(t_sh[:], t_pdf[:], float(C1), None, OP.subtract)
            p_S = pse.tile([2, 256], F32, name="p_S", tag="pS")
            nc.tensor.matmul(p_S[:, 0:128], t_mask[:], t_sh[:], start=True, stop=True)
            t_S = wk.tile([2, 128], F32, name="t_S")
            nc.vector.tensor_scalar(t_S[:], p_S[:, 0:128], C1_64, None, OP.add)
            t_rS = wk.tile([2, 128], F32, name="t_rS")
            nc.vector.reciprocal(t_rS[:], t_S[:])
            t_nS = wk.tile([2, 128], F32, name="t_nS")
            nc.vector.tensor_scalar(t_nS[:], t_S[:], -1.0, None, OP.mult)
            # broadcast rS, nS to [128,128]
            p_b = pse.tile([128, 512], F32, name="p_b", tag="plate")
            nc.tensor.matmul(p_b[:, 0:128], t_maskT[:], t_rS[:], start=True, stop=True)
            nc.tensor.matmul(p_b[:, 128:256], t_maskT[:], t_nS[:], start=True, stop=True)
            # Newton divide q = pdf/S
            t_qq0 = wk.tile([128, 128], F32, name="t_qq0")
            nc.vector.tensor_tensor(t_qq0[:], t_pdf[:], p_b[:, 0:128], OP.mult)
            t_t1 = wk.tile([128, 128], F32, name="t_t1")
            nc.vector.tensor_tensor(t_t1[:], t_qq0[:], p_b[:, 128:256], OP.mult)
            t_rn = wk.tile([128, 128], F32, name="t_rn")
            nc.vector.tensor_tensor(t_rn[:], t_t1[:], t_pdf[:], OP.add)
            t_t2 = wk.tile([128, 128], F32, name="t_t2")
            nc.vector.tensor_tensor(t_t2[:], t_rn[:], p_b[:, 0:128], OP.mult)
            t_q = wk.tile([128, 128], F32, name="t_q")
            nc.vector.tensor_tensor(t_q[:], t_t2[:], t_qq0[:], OP.add)
            nc.vector.tensor_scalar(t_q[:], t_q[:], 1e-19, None, OP.max)
            t_ln = wk.tile([128, 128], F32, name="t_ln")
            nc.scalar.activation(t_ln[:], t_q[:], AF.Ln, bias=0.0, scale=1.0)
            t_pr = wk.tile([128, 128], F32, name="t_pr")
            nc.vector.tensor_tensor(t_pr[:], t_q[:], t_ln[:], OP.mult)
            t_sh2 = wk.tile([128, 128], F32, name="t_sh2")
            nc.vector.tensor_scalar(t_sh2[:], t_pr[:], float(C2), None, OP.add)
            nc.tensor.matmul(p_S[:, 128:256], t_mask[:], t_sh2[:], start=True, stop=True)
            t_e = wk.tile([2, 128], F32, name="t_e")
            nc.vector.tensor_scalar(t_e[:], p_S[:, 128:256], -1.0, C2_64, OP.mult, OP.add)
            nc.sync.dma_start(ent_out.ap(), t_e[:])
            # row r = 2t + hh -> ag_in[0, r]
            ag_ap = bass.AP(ag_in, 0, [[1, 2], [2, 128]])
            nc.sync.dma_start(ag_ap, t_e[:])

            # ---- AllGather entropies within sample pairs ----
            nc.gpsimd.collective_compute(
                "AllGather", OP.bypass,
                replica_groups=[[0, 1], [2, 3], [4, 5], [6, 7]],
                ins=[ag_in.ap()], outs=[ag_out.ap()],
            )

            # ---- rank stage ----
            t_eall = wk.tile([1, L], F32, name="t_eall")
            nc.sync.dma_start(t_eall[:], ag_out.ap())
            # broadcast eall to 128 partitions (f32 matmul, exact)
            p_e = pse.tile([128, 512], F32, name="p_e", tag="plate")
            nc.tensor.matmul(p_e[:], t_ones1[:], t_eall[:], start=True, stop=True)
            t_iota = wk.tile([128, L], I32, name="t_iota")
            nc.gpsimd.iota(t_iota[:], pattern=[[1, L]], base=0, channel_multiplier=0)
            t_iotaf = wk.tile([128, L], F32, name="t_iotaf")
            nc.vector.tensor_copy(t_iotaf[:], t_iota[:])
            t_pidx = wk.tile([128, 1], I32, name="t_pidx")
            nc.gpsimd.iota(t_pidx[:], pattern=[[0, 1]], base=0, channel_multiplier=1)
            t_pidxf = wk.tile([128, 1], F32, name="t_pidxf")
            nc.vector.tensor_copy(t_pidxf[:], t_pidx[:])

            t_hbb = wk.tile([128, 1], F32, name="t_hbb")
            nc.gpsimd.partition_broadcast(t_hbb[:], t_hb[:], channels=128)
            for ci in range(2):
                # e_me[p] = my row (ci*128+p)'s entropy, from ag_in (row order)
                t_eme = wk.tile([128, 1], F32, name=f"t_eme{ci}")
                nc.sync.dma_start(
                    t_eme[:], bass.AP(ag_in, ci * 128, [[1, 128], [1, 1]]))
                # global row index threshold for stable tie-break
                t_thrf = wk.tile([128, 1], F32, name=f"t_thrf{ci}")
                nc.vector.tensor_tensor(t_thrf[:], t_pidxf[:], t_hbb[:], OP.add)
                if ci:
                    nc.vector.tensor_scalar(t_thrf[:], t_thrf[:], 128.0, None, OP.add)
                t_lt = wk.tile([128, L], F32, name=f"t_lt{ci}")
                nc.vector.tensor_scalar(t_lt[:], p_e[:], t_eme[:], None, OP.is_lt)
                t_eq = wk.tile([128, L], F32, name=f"t_eq{ci}")
                nc.vector.tensor_scalar(t_eq[:], p_e[:], t_eme[:], None, OP.is_equal)
                t_il = wk.tile([128, L], F32, name=f"t_il{ci}")
                nc.vector.tensor_scalar(t_il[:], t_iotaf[:], t_thrf[:], None, OP.is_lt)
                t_m1 = wk.tile([128, L], F32, name=f"t_m1{ci}")
                nc.vector.tensor_tensor(t_m1[:], t_eq[:], t_il[:], OP.mult)
                t_dm = wk.tile([128, L], F32, name=f"t_dm{ci}")
                t_rkf = wk.tile([128, 1], F32, name=f"t_rkf{ci}")
                nc.vector.scalar_tensor_tensor(
                    t_dm[:], t_m1[:], 0.0, t_lt[:], op0=OP.add, op1=OP.add,
                    accum_out=t_rkf[:])
                t_rki = wk.tile([128, 1], I32, name=f"t_rki{ci}")
                nc.vector.tensor_copy(t_rki[:], t_rkf[:])
                nc.sync.dma_start(
                    bass.AP(idr_out, ci * 128, [[1, 128], [1, 1]]),
                    t_rki[:])
                t_msk = wk.tile([128, 1], F32, name=f"t_msk{ci}")
                nc.vector.tensor_scalar(t_msk[:], t_rkf[:], 127.5, None, OP.is_ge)
                nc.sync.dma_start(
                    bass.AP(mask_out, ci * 128, [[1, 128], [1, 1]]),
                    t_msk[:])
                # scatter kept x rows
                nc.gpsimd.indirect_dma_start(
                    xm_out.ap(),
                    IndirectOffsetOnAxis(ap=t_rki[:], axis=0),
                    t_x[:, ci * D:(ci + 1) * D],
                    None,
                    bounds_check=LK - 1,
                    oob_is_err=False,
                )

    nc.compile()
    _NC_CACHE["nc"] = nc
    return nc


def kernel(x, img_pat):
    x = np.ascontiguousarray(x, dtype=np.float32)
    img_pat = np.ascontiguousarray(img_pat, dtype=np.float32)
    nc = _build()

    gimg_full = np.ascontiguousarray(
        img_pat.reshape(8, 128, 12288 // 8).transpose(1, 0, 2).reshape(128, 12288))
    in_maps = []
    for c in range(8):
        n, h = c // 2, c % 2
        ip_rows = img_pat[n, h * RPC:(h + 1) * RPC]          # [256, 768]
        x_rows = x[n, h * RPC:(h + 1) * RPC]                 # [256, 1024]
        in_maps.append(dict(
            ipat=np.ascontiguousarray(
                ip_rows.reshape(2, 128, P).transpose(1, 0, 2).reshape(128, 2 * P)),
            xrows=np.ascontiguousarray(
                x_rows.reshape(2, 128, D).transpose(1, 0, 2).reshape(128, 2 * D)),
            hbase=np.array([[h * RPC]], np.float32),
            gimg=gimg_full,
        ))

    kw = {}
    if os.environ.get("KERNEL_TRACE"):
        kw = dict(trace=True, trace_cores=[0])
    res = run_bass_kernel_spmd(nc, in_maps, core_ids=list(range(8)), **kw)
    LAST_RESULT["res"] = res

    x_masked = np.zeros((N, LK, D), np.float32)
    mask = np.zeros((N, L), np.float32)
    idr = np.zeros((N, L), np.int32)
    for c in range(8):
        n, h = c // 2, c % 2
        o = res.results[c]
        x_masked[n] += o["xm_out"]
        mask[n, h * RPC:(h + 1) * RPC] = o["mask_out"][0]
        idr[n, h * RPC:(h + 1) * RPC] = o["idr_out"][0]
    return x_masked, mask, idr


if __name__ == "__main__":
    import reference
    inputs = {k: np.asarray(v) for k, v in reference.setup_inputs().items()}
    out = kernel(**inputs)
    print([o.shape for o in out])
